# revision 1
# baseline (speedup 1.0000x reference)
"""Multi-head attention (B=2, S=2048, D=1024, H=16) on 8 NeuronCores.

Sharding: Megatron tensor parallelism. Core r owns heads 2r, 2r+1
(a 128-wide slice of D). Wq/Wk/Wv column-parallel, Wo row-parallel,
ReduceScatter(add) over tokens at the end; host concatenates the 8
token slices and adds bo.

Layouts on device (per core):
  xqT/xkT/xvT : [1024, 4096]  host-transposed activations (feature-major)
  qT/kT       : [128, 2048]   per batch, dk-major (rows = this core's 2 heads)
  v           : [128, 130]    16 token-tiles per batch; cols = [v_h0 | 1 | v_h1 | 1]
                              (ones column makes the PV matmul emit softmax sums)
  scores^T    : psum [128 sk, 512 sq] -> exp on ACT -> PT sbuf
  PV          : psum [65, 512] accumulated over 16 sk tiles; row 64 = sums
  attnT       : [128, 2048]   per batch, normalized, = lhsT for Wo matmul
"""

import sys

sys.path.insert(0, "/opt/trn_rl_repo")

import numpy as np

B, S, D, H, DK = 2, 2048, 1024, 16, 64
NCORES = 8
TOK = B * S            # 4096
DKC = D // NCORES      # 128 = 2 heads per core
TOKC = TOK // NCORES   # 512 output rows per core
KT = D // 128          # 8 contraction tiles
SKT = S // 128         # 16 key tiles per batch
SQB = S // 512         # 4 query blocks per batch

# matmul operand dtype: float32 (exact, 4 cyc/row) or float32r (1 cyc/row)
MM_DT_NAME = "float32r"

_cache = {}


def _build(collective=True):
    from contextlib import ExitStack

    from concourse import bacc
    import concourse.mybir as mybir
    import concourse.tile as tile

    f32 = mybir.dt.float32
    mm_dt = getattr(mybir.dt, MM_DT_NAME)
    Act = mybir.ActivationFunctionType

    def c(ap):
        # bitcast DRAM sources feeding matmul-operand tiles to the matmul dtype
        return ap.bitcast(mm_dt) if mm_dt != f32 else ap

    nc = bacc.Bacc(
        "TRN2", target_bir_lowering=False, debug=False,
        enable_asserts=False, num_devices=NCORES,
    )

    xqT = nc.dram_tensor("xqT", [D, TOK], f32, kind="ExternalInput").ap()
    xkT = nc.dram_tensor("xkT", [D, TOK], f32, kind="ExternalInput").ap()
    xvT = nc.dram_tensor("xvT", [D, TOK], f32, kind="ExternalInput").ap()
    wq = nc.dram_tensor("wq", [D, DKC], f32, kind="ExternalInput").ap()
    wk = nc.dram_tensor("wk", [D, DKC], f32, kind="ExternalInput").ap()
    wv = nc.dram_tensor("wv", [D, DKC], f32, kind="ExternalInput").ap()
    wo = nc.dram_tensor("wo", [DKC, D], f32, kind="ExternalInput").ap()
    bq = nc.dram_tensor("bq", [DKC, 1], f32, kind="ExternalInput").ap()
    bk = nc.dram_tensor("bk", [DKC, 1], f32, kind="ExternalInput").ap()
    bv = nc.dram_tensor("bv", [1, DKC], f32, kind="ExternalInput").ap()
    out_ext = nc.dram_tensor("out", [TOKC, D], f32, kind="ExternalOutput").ap()

    with tile.TileContext(nc) as tc, ExitStack() as ctx, \
            nc.allow_low_precision("float32r matmul operands, fp32 psum accumulate"):
        wpool = ctx.enter_context(tc.tile_pool(name="w", bufs=1))
        xpool = ctx.enter_context(tc.tile_pool(name="x", bufs=12))
        qkpool = ctx.enter_context(tc.tile_pool(name="qk", bufs=2))
        vpool = ctx.enter_context(tc.tile_pool(name="v", bufs=32))
        ptpool = ctx.enter_context(tc.tile_pool(name="pt", bufs=6))
        atpool = ctx.enter_context(tc.tile_pool(name="at", bufs=2))
        smpool = ctx.enter_context(tc.tile_pool(name="sm", bufs=4))
        opool = ctx.enter_context(tc.tile_pool(name="o", bufs=4))
        ps_mm = ctx.enter_context(tc.tile_pool(name="psmm", bufs=3, space="PSUM"))
        ps_acc = ctx.enter_context(tc.tile_pool(name="psacc", bufs=2, space="PSUM"))
        dram = ctx.enter_context(tc.tile_pool(name="dram", bufs=1, space="DRAM"))

        # ---- constants / weights into SBUF ----
        wq_t, wk_t, wv_t = [], [], []
        for name, src, lst in (("wq", wq, wq_t), ("wk", wk, wk_t), ("wv", wv, wv_t)):
            for k in range(KT):
                t = wpool.tile([128, DKC], mm_dt, tag=f"{name}{k}")
                nc.sync.dma_start(t[:], c(src[k * 128:(k + 1) * 128, :]))
                lst.append(t)
        wo_t = wpool.tile([DKC, D], mm_dt, tag="wo")
        nc.sync.dma_start(wo_t[:], c(wo[:]))
        bq_t = wpool.tile([DKC, 1], f32, tag="bq")
        nc.sync.dma_start(bq_t[:], bq[:])
        bk_t = wpool.tile([DKC, 1], f32, tag="bk")
        nc.sync.dma_start(bk_t[:], bk[:])
        bv_t = wpool.tile([1, DKC], mm_dt, tag="bv")
        nc.sync.dma_start(bv_t[:], c(bv[:]))
        ones_f = wpool.tile([1, 128], f32, tag="onesf")
        nc.gpsimd.memset(ones_f[:], 1.0)
        ones_t = wpool.tile([1, 128], mm_dt, tag="ones")
        nc.vector.tensor_copy(ones_t[:], ones_f[:])
        onescol_f = wpool.tile([128, 1], f32, tag="onescolf")
        nc.gpsimd.memset(onescol_f[:], 1.0)

        partial = dram.tile([TOK, D], f32, tag="partial")
        rs_out = dram.tile([TOKC, D], f32, tag="rsout")

        for b in range(B):
            t0 = b * S
            # ---- q/k projections -> qT_b, kT_b [128, S] (dk-major) ----
            qT_b = qkpool.tile([128, S], mm_dt, tag="qT")
            kT_b = qkpool.tile([128, S], mm_dt, tag="kT")
            for xT, w_list, bias_t, dst in (
                (xqT, wq_t, bq_t, qT_b), (xkT, wk_t, bk_t, kT_b),
            ):
                for blk in range(SQB):
                    ps = ps_mm.tile([128, 512], f32, tag="mm")
                    for k in range(KT):
                        xt = xpool.tile([128, 512], mm_dt, tag="xt")
                        nc.sync.dma_start(
                            xt[:],
                            c(xT[k * 128:(k + 1) * 128,
                                 t0 + blk * 512: t0 + (blk + 1) * 512]),
                        )
                        nc.tensor.matmul(
                            ps[:], lhsT=w_list[k][:], rhs=xt[:],
                            start=(k == 0), stop=(k == KT - 1),
                        )
                    nc.scalar.activation(
                        dst[:, blk * 512:(blk + 1) * 512], ps[:],
                        Act.Identity, bias=bias_t[:, 0:1],
                    )

            # ---- v projection -> 16 tiles [128 tok, 130] ----
            v_tiles = []
            for blk in range(SQB):
                xv_blk = []
                for k in range(KT):
                    xt = xpool.tile([128, 512], mm_dt, tag="xt")
                    nc.sync.dma_start(
                        xt[:],
                        c(xvT[k * 128:(k + 1) * 128,
                              t0 + blk * 512: t0 + (blk + 1) * 512]),
                    )
                    xv_blk.append(xt)
                for mi in range(4):
                    ps = ps_mm.tile([128, DKC], f32, tag="mm")
                    for k in range(KT):
                        nc.tensor.matmul(
                            ps[:], lhsT=xv_blk[k][:, mi * 128:(mi + 1) * 128],
                            rhs=wv_t[k][:], start=(k == 0), stop=False,
                        )
                    nc.tensor.matmul(
                        ps[:], lhsT=ones_t[0:1, :], rhs=bv_t[:],
                        start=False, stop=True,
                    )
                    vt = vpool.tile([128, 130], mm_dt, tag="v")
                    nc.vector.tensor_copy(vt[:, 0:64], ps[:, 0:64])
                    nc.vector.tensor_copy(vt[:, 65:129], ps[:, 64:128])
                    nc.vector.tensor_copy(vt[:, 64:65], onescol_f[:])
                    nc.vector.tensor_copy(vt[:, 129:130], onescol_f[:])
                    v_tiles.append(vt)

            # ---- attention (2 heads) -> attnT_b [128, S] ----
            attnT_b = atpool.tile([128, S], mm_dt, tag="attnT")
            for h in range(2):
                hp = h * 64
                for sq in range(SQB):
                    qs = slice(sq * 512, (sq + 1) * 512)
                    xps = ps_acc.tile([65, 512], f32, tag="acc")
                    for sk in range(SKT):
                        sps = ps_mm.tile([128, 512], f32, tag="mm")
                        nc.tensor.matmul(
                            sps[:],
                            lhsT=kT_b[hp:hp + 64, sk * 128:(sk + 1) * 128],
                            rhs=qT_b[hp:hp + 64, qs],
                            start=True, stop=True,
                        )
                        pt = ptpool.tile([128, 512], mm_dt, tag="pt")
                        nc.scalar.activation(pt[:], sps[:], Act.Exp, scale=0.125)
                        nc.tensor.matmul(
                            xps[:], lhsT=v_tiles[sk][:, h * 65:h * 65 + 65],
                            rhs=pt[:], start=(sk == 0), stop=(sk == SKT - 1),
                        )
                    rec = smpool.tile([1, 512], mm_dt, tag="rec")
                    nc.vector.reciprocal(rec[:], xps[64:65, :])
                    rbp = ps_mm.tile([64, 512], f32, tag="mm")
                    nc.tensor.matmul(
                        rbp[:], lhsT=ones_t[0:1, 0:64], rhs=rec[:],
                        start=True, stop=True,
                    )
                    rb = smpool.tile([64, 512], f32, tag="rb")
                    nc.scalar.copy(rb[:], rbp[:])
                    nc.vector.tensor_mul(
                        attnT_b[hp:hp + 64, qs], xps[0:64, :], rb[:],
                    )

            # ---- output projection partial [S, D] ----
            for m in range(S // 128):
                for n2 in range(2):
                    ops = ps_mm.tile([128, 512], f32, tag="mm")
                    nc.tensor.matmul(
                        ops[:], lhsT=attnT_b[:, m * 128:(m + 1) * 128],
                        rhs=wo_t[:, n2 * 512:(n2 + 1) * 512],
                        start=True, stop=True,
                    )
                    ot = opool.tile([128, 512], f32, tag="ot")
                    nc.vector.tensor_copy(ot[:], ops[:])
                    nc.sync.dma_start(
                        partial[t0 + m * 128: t0 + (m + 1) * 128,
                                n2 * 512:(n2 + 1) * 512],
                        ot[:],
                    )

        if collective:
            nc.gpsimd.collective_compute(
                "ReduceScatter",
                mybir.AluOpType.add,
                replica_groups=[list(range(NCORES))],
                ins=[partial.opt()],
                outs=[rs_out.opt()],
            )
            nc.sync.dma_start(out_ext[:], rs_out[:])
        else:
            nc.sync.dma_start(out_ext[:], partial[0:TOKC, :])

    nc.compile()
    return nc


def _get_nc():
    if "nc" not in _cache:
        _cache["nc"] = _build()
    return _cache["nc"]


def kernel(query, key, value, Wq, bq, Wk, bk, Wv, bv, Wo, bo, trace=False):
    from concourse.bass_utils import run_bass_kernel_spmd

    nc = _get_nc()

    q = np.ascontiguousarray(np.asarray(query, np.float32).reshape(TOK, D).T)
    k = np.ascontiguousarray(np.asarray(key, np.float32).reshape(TOK, D).T)
    v = np.ascontiguousarray(np.asarray(value, np.float32).reshape(TOK, D).T)
    Wq = np.asarray(Wq, np.float32)
    Wk = np.asarray(Wk, np.float32)
    Wv = np.asarray(Wv, np.float32)
    Wo = np.asarray(Wo, np.float32)

    in_maps = []
    for r in range(NCORES):
        sl = slice(r * DKC, (r + 1) * DKC)
        in_maps.append({
            "xqT": q, "xkT": k, "xvT": v,
            "wq": np.ascontiguousarray(Wq[:, sl]),
            "wk": np.ascontiguousarray(Wk[:, sl]),
            "wv": np.ascontiguousarray(Wv[:, sl]),
            "wo": np.ascontiguousarray(Wo[sl, :]),
            "bq": np.ascontiguousarray(np.asarray(bq, np.float32)[sl, None]),
            "bk": np.ascontiguousarray(np.asarray(bk, np.float32)[sl, None]),
            "bv": np.ascontiguousarray(np.asarray(bv, np.float32)[None, sl]),
        })

    res = run_bass_kernel_spmd(nc, in_maps, list(range(NCORES)), trace=trace)
    _cache["last_results"] = res

    out = np.concatenate([res.results[r]["out"] for r in range(NCORES)], axis=0)
    out = out + np.asarray(bo, np.float32)[None, :]
    return out.reshape(B, S, D)



# revision 5
# speedup vs baseline: 2.2927x; 2.2927x over previous
"""Multi-head attention (B=2, S=2048, D=1024, H=16) on 8 NeuronCores.

Sharding: Megatron tensor parallelism. Core r owns heads 2r, 2r+1
(a 128-wide slice of D). Wq/Wk/Wv column-parallel, Wo row-parallel,
chunked ReduceScatter(add) over tokens at the end; host reassembles
the token chunks and adds bo.

All matmul operands are bf16 (fp32 PSUM accumulate). Host pre-casts
activations/weights to bf16 and pre-transposes x to feature-major.

Per-core layouts:
  xqT/xkT/xvT : [1024, 4096] bf16  feature-major activations
  qT/kT       : [128, 2048] per batch; rows 0:64 = head0 dk, 64:128 = head1
  v           : [128, 130] x16 per batch; cols = [v_h0 | 1 | v_h1 | 1]
                (ones columns make the PV matmul emit softmax sums)
  scores      : psum [128 sk, 1024] = [h0 block | h1 block]; the two score
                matmuls run CONCURRENTLY via PE row tiling (K=64: h0 in
                array rows 0-63, h1 in rows 64-127)
  exp         : one ACT instr per [128, 1024] psum tile -> pt bf16 sbuf
  PV          : psum [65, 512] per head accumulated over 16 sk tiles;
                row 64 = softmax sums
  attnT       : [128, 2048] per batch, normalized, dk-major
  out-proj    : partial [tok, 1024] bf16 -> DRAM, ReduceScatter per
                1024-token chunk (4 chunks), overlapped with compute

The emission is software-pipelined: each (sq, sk) attention iteration
also pops one deferred thunk (previous block's normalization/out-proj,
or the next batch's projections) so the PE never drains while the ACT
engine works through the exp stream.
"""

import sys

sys.path.insert(0, "/opt/trn_rl_repo")

import numpy as np

B, S, D, H, DK = 2, 2048, 1024, 16, 64
NCORES = 8
TOK = B * S            # 4096
DKC = D // NCORES      # 128 = 2 heads per core
KT = D // 128          # 8 contraction tiles
SKT = S // 128         # 16 key tiles per batch
SQB = S // 512         # 4 query blocks per batch
NCHUNK = 4             # ReduceScatter chunks of 1024 tokens each

_cache = {}


def _build(collective=True):
    from contextlib import ExitStack

    from concourse import bacc
    import concourse.mybir as mybir
    import concourse.tile as tile

    f32 = mybir.dt.float32
    bf16 = mybir.dt.bfloat16
    Act = mybir.ActivationFunctionType

    nc = bacc.Bacc(
        "TRN2", target_bir_lowering=False, debug=False,
        enable_asserts=False, num_devices=NCORES,
    )

    xqT = nc.dram_tensor("xqT", [D, TOK], bf16, kind="ExternalInput").ap()
    xkT = nc.dram_tensor("xkT", [D, TOK], bf16, kind="ExternalInput").ap()
    xvT = nc.dram_tensor("xvT", [D, TOK], bf16, kind="ExternalInput").ap()
    wq = nc.dram_tensor("wq", [D, DKC], bf16, kind="ExternalInput").ap()
    wk = nc.dram_tensor("wk", [D, DKC], bf16, kind="ExternalInput").ap()
    wv = nc.dram_tensor("wv", [D, DKC], bf16, kind="ExternalInput").ap()
    wo = nc.dram_tensor("wo", [DKC, D], bf16, kind="ExternalInput").ap()
    bq = nc.dram_tensor("bq", [DKC, 1], f32, kind="ExternalInput").ap()
    bk = nc.dram_tensor("bk", [DKC, 1], f32, kind="ExternalInput").ap()
    bv = nc.dram_tensor("bv", [1, DKC], bf16, kind="ExternalInput").ap()
    out_ext = nc.dram_tensor("out", [NCHUNK * 128, D], bf16,
                             kind="ExternalOutput").ap()

    with tile.TileContext(nc) as tc, ExitStack() as ctx, \
            nc.allow_low_precision("bf16 matmul operands, fp32 psum accumulate"):
        wpool = ctx.enter_context(tc.tile_pool(name="w", bufs=1))
        xpool = ctx.enter_context(tc.tile_pool(name="x", bufs=12))
        qkpool = ctx.enter_context(tc.tile_pool(name="qk", bufs=2))
        vpool = ctx.enter_context(tc.tile_pool(name="v", bufs=32))
        ptpool = ctx.enter_context(tc.tile_pool(name="pt", bufs=4))
        atpool = ctx.enter_context(tc.tile_pool(name="at", bufs=2))
        smpool = ctx.enter_context(tc.tile_pool(name="sm", bufs=2))
        opool = ctx.enter_context(tc.tile_pool(name="o", bufs=4))
        # PSUM budget (8 banks): sc 2x[128,1024]=4, acc0+acc1=2, gen 2x[128,512]=2
        ps_sc = ctx.enter_context(tc.tile_pool(name="pssc", bufs=2, space="PSUM"))
        ps_acc = ctx.enter_context(tc.tile_pool(name="psacc", bufs=1, space="PSUM"))
        ps_gen = ctx.enter_context(tc.tile_pool(name="psgen", bufs=2, space="PSUM"))
        dram = ctx.enter_context(tc.tile_pool(name="dram", bufs=1, space="DRAM"))

        # ---- constants / weights into SBUF ----
        wq_t, wk_t, wv_t = [], [], []
        for name, src, lst in (("wq", wq, wq_t), ("wk", wk, wk_t), ("wv", wv, wv_t)):
            for k in range(KT):
                t = wpool.tile([128, DKC], bf16, tag=f"{name}{k}")
                nc.sync.dma_start(t[:], src[k * 128:(k + 1) * 128, :])
                lst.append(t)
        wo_t = wpool.tile([DKC, D], bf16, tag="wo")
        nc.sync.dma_start(wo_t[:], wo[:])
        bq_t = wpool.tile([DKC, 1], f32, tag="bq")
        nc.sync.dma_start(bq_t[:], bq[:])
        bk_t = wpool.tile([DKC, 1], f32, tag="bk")
        nc.sync.dma_start(bk_t[:], bk[:])
        bv_t = wpool.tile([1, DKC], bf16, tag="bv")
        nc.sync.dma_start(bv_t[:], bv[:])
        ones_t = wpool.tile([1, 128], bf16, tag="ones")
        nc.vector.memset(ones_t[:], 1.0)

        partials = [dram.tile([1024, D], bf16, tag=f"partial{c}",
                              name=f"partial{c}") for c in range(NCHUNK)]
        rs_outs = [dram.tile([128, D], bf16, tag=f"rsout{c}",
                             name=f"rsout{c}") for c in range(NCHUNK)]

        # ---------- emission helpers ----------

        def load_x(xT, k, b, blk):
            """DMA one [128, 512] slice of an activation tensor."""
            xt = xpool.tile([128, 512], bf16, tag="xt")
            c0 = b * S + blk * 512
            nc.sync.dma_start(xt[:], xT[k * 128:(k + 1) * 128, c0:c0 + 512])
            return xt

        def qk_block(xT, w_list, bias_t, dst, b, blk):
            """One 512-col block of a q/k projection -> dst[:, blk]."""
            xts = [load_x(xT, k, b, blk) for k in range(KT)]
            ps = ps_gen.tile([128, 512], f32, tag="gen")
            for k in range(KT):
                nc.tensor.matmul(
                    ps[:], lhsT=w_list[k][:], rhs=xts[k][:],
                    start=(k == 0), stop=(k == KT - 1),
                )
            nc.vector.tensor_scalar_add(
                dst[:, blk * 512:(blk + 1) * 512], ps[:], bias_t[:, 0:1])

        def v_block(b, blk, v_tiles):
            """Four token-tiles [128, 130] of the v projection."""
            xts = [load_x(xvT, k, b, blk) for k in range(KT)]
            for mi in range(4):
                ps = ps_gen.tile([128, 512], f32, tag="gen")
                for k in range(KT):
                    nc.tensor.matmul(
                        ps[:, 0:128],
                        lhsT=xts[k][:, mi * 128:(mi + 1) * 128],
                        rhs=wv_t[k][:], start=(k == 0), stop=False,
                    )
                nc.tensor.matmul(
                    ps[:, 0:128], lhsT=ones_t[0:1, :], rhs=bv_t[:],
                    start=False, stop=True,
                )
                vt = vpool.tile([128, 130], bf16, tag="v")
                nc.vector.tensor_copy(vt[:, 0:64], ps[:, 0:64])
                nc.vector.tensor_copy(vt[:, 65:129], ps[:, 64:128])
                nc.vector.memset(vt[:, 64:65], 1.0)
                nc.vector.memset(vt[:, 129:130], 1.0)
                v_tiles.append(vt)

        def emit_attn_iter(qT_b, kT_b, v_tiles, accs, sq, sk):
            """One (sq, sk) step: packed scores pair, one exp, two PV MMs."""
            qs = slice(sq * 512, (sq + 1) * 512)
            ks = slice(sk * 128, (sk + 1) * 128)
            sps = ps_sc.tile([128, 1024], f32, tag="sc")
            # two heads run concurrently in PE rows 0-63 / 64-127
            nc.tensor.matmul(
                sps[:, 0:512], lhsT=kT_b[0:64, ks], rhs=qT_b[0:64, qs],
                start=True, stop=True,
            )
            nc.tensor.matmul(
                sps[:, 512:1024], lhsT=kT_b[64:128, ks], rhs=qT_b[64:128, qs],
                start=True, stop=True,
            )
            pt = ptpool.tile([128, 1024], bf16, tag="pt")
            nc.scalar.activation(pt[:], sps[:], Act.Exp, scale=0.125)
            nc.tensor.matmul(
                accs[0][:], lhsT=v_tiles[sk][:, 0:65], rhs=pt[:, 0:512],
                start=(sk == 0), stop=(sk == SKT - 1),
            )
            nc.tensor.matmul(
                accs[1][:], lhsT=v_tiles[sk][:, 65:130], rhs=pt[:, 512:1024],
                start=(sk == 0), stop=(sk == SKT - 1),
            )

        def drain_accs(accs):
            """Inline epilogue part 1 (DVE only): pull reciprocal seeds and
            raw attention out of PSUM so the acc banks free up fast."""
            rec = smpool.tile([1, 1024], bf16, tag="rec")
            araw = smpool.tile([128, 512], bf16, tag="araw")
            nc.vector.reciprocal(rec[0:1, 0:512], accs[0][64:65, :])
            nc.vector.reciprocal(rec[0:1, 512:1024], accs[1][64:65, :])
            nc.vector.tensor_copy(araw[0:64, :], accs[0][0:64, :])
            nc.vector.tensor_copy(araw[64:128, :], accs[1][0:64, :])
            return rec, araw

        def make_epilogue(attnT_b, rec, araw, b, sq):
            """Deferred epilogue thunks for (b, sq): broadcast-normalize,
            out-projection, and (on odd sq) the chunk's ReduceScatter."""
            qs = slice(sq * 512, (sq + 1) * 512)
            state = {}

            def s_norm():
                rb_ps = ps_gen.tile([128, 512], f32, tag="gen")
                # the two broadcast matmuls run concurrently (col tiling)
                nc.tensor.matmul(
                    rb_ps[0:64, :], lhsT=ones_t[0:1, 0:64],
                    rhs=rec[0:1, 0:512], start=True, stop=True,
                )
                nc.tensor.matmul(
                    rb_ps[64:128, :], lhsT=ones_t[0:1, 0:64],
                    rhs=rec[0:1, 512:1024], start=True, stop=True,
                )
                rb = smpool.tile([128, 512], bf16, tag="rb")
                nc.vector.tensor_copy(rb[:], rb_ps[:])
                nc.vector.tensor_mul(attnT_b[:, qs], araw[:], rb[:])

            def out_m(m):
                def f():
                    col = sq * 512 + m * 128
                    chunk = b * 2 + sq // 2
                    row0 = (sq % 2) * 512 + m * 128
                    for n2 in range(2):
                        ops = ps_gen.tile([128, 512], f32, tag="gen")
                        nc.tensor.matmul(
                            ops[:], lhsT=attnT_b[:, col:col + 128],
                            rhs=wo_t[:, n2 * 512:(n2 + 1) * 512],
                            start=True, stop=True,
                        )
                        ot = opool.tile([128, 512], bf16, tag="ot")
                        nc.vector.tensor_copy(ot[:], ops[:])
                        nc.sync.dma_start(
                            partials[chunk][row0:row0 + 128,
                                            n2 * 512:(n2 + 1) * 512],
                            ot[:],
                        )
                return f

            thunks = [s_norm] + [out_m(m) for m in range(4)]
            if sq % 2 == 1 and collective:
                chunk = b * 2 + sq // 2

                def rs():
                    nc.gpsimd.collective_compute(
                        "ReduceScatter",
                        mybir.AluOpType.add,
                        replica_groups=[list(range(NCORES))],
                        ins=[partials[chunk].opt()],
                        outs=[rs_outs[chunk].opt()],
                    )
                    nc.sync.dma_start(
                        out_ext[chunk * 128:(chunk + 1) * 128, :],
                        rs_outs[chunk][:],
                    )
                thunks.append(rs)
            return thunks

        # ---------- main emission ----------
        qT, kT, attnT = {}, {}, {}
        vt_all = {0: [], 1: []}

        for b in (0, 1):
            kT[b] = qkpool.tile([128, S], bf16, tag="kT", name=f"kT{b}")
            qT[b] = qkpool.tile([128, S], bf16, tag="qT", name=f"qT{b}")
            attnT[b] = atpool.tile([128, S], bf16, tag="attnT", name=f"attnT{b}")

        # batch 0 projections emitted up front (k fully, then q, then v)
        for blk in range(SQB):
            qk_block(xkT, wk_t, bk_t, kT[0], 0, blk)
        for blk in range(SQB):
            qk_block(xqT, wq_t, bq_t, qT[0], 0, blk)
        for blk in range(SQB):
            v_block(0, blk, vt_all[0])

        # batch 1 projections become thunks interleaved into b0 attention
        prelude = []
        for blk in range(SQB):
            prelude.append(lambda blk=blk: qk_block(xkT, wk_t, bk_t, kT[1], 1, blk))
        for blk in range(SQB):
            prelude.append(lambda blk=blk: qk_block(xqT, wq_t, bq_t, qT[1], 1, blk))
        for blk in range(SQB):
            prelude.append(lambda blk=blk: v_block(1, blk, vt_all[1]))

        pending = []

        def emit_batch_attention(b, extra):
            for sq in range(SQB):
                accs = (
                    ps_acc.tile([65, 512], f32, tag="acc0", name="acc0"),
                    ps_acc.tile([65, 512], f32, tag="acc1", name="acc1"),
                )
                for sk in range(SKT):
                    emit_attn_iter(qT[b], kT[b], vt_all[b], accs, sq, sk)
                    if pending:
                        pending.pop(0)()
                    elif extra:
                        extra.pop(0)()
                rec, araw = drain_accs(accs)
                pending.extend(make_epilogue(attnT[b], rec, araw, b, sq))
            while extra:
                extra.pop(0)()

        emit_batch_attention(0, prelude)
        emit_batch_attention(1, [])
        for f in pending:
            f()
        pending.clear()

        if not collective:
            for c in range(NCHUNK):
                nc.sync.dma_start(
                    out_ext[c * 128:(c + 1) * 128, :],
                    partials[c][0:128, :],
                )

    nc.compile()
    return nc


def _get_nc():
    if "nc" not in _cache:
        _cache["nc"] = _build()
    return _cache["nc"]


def kernel(query, key, value, Wq, bq, Wk, bk, Wv, bv, Wo, bo, trace=False):
    from concourse.bass_utils import run_bass_kernel_spmd
    import ml_dtypes

    bfloat16 = ml_dtypes.bfloat16
    nc = _get_nc()

    q = np.ascontiguousarray(
        np.asarray(query, np.float32).reshape(TOK, D).T.astype(bfloat16))
    k = np.ascontiguousarray(
        np.asarray(key, np.float32).reshape(TOK, D).T.astype(bfloat16))
    v = np.ascontiguousarray(
        np.asarray(value, np.float32).reshape(TOK, D).T.astype(bfloat16))
    Wq = np.asarray(Wq, np.float32)
    Wk = np.asarray(Wk, np.float32)
    Wv = np.asarray(Wv, np.float32)
    Wo = np.asarray(Wo, np.float32)

    in_maps = []
    for r in range(NCORES):
        sl = slice(r * DKC, (r + 1) * DKC)
        in_maps.append({
            "xqT": q, "xkT": k, "xvT": v,
            "wq": np.ascontiguousarray(Wq[:, sl].astype(bfloat16)),
            "wk": np.ascontiguousarray(Wk[:, sl].astype(bfloat16)),
            "wv": np.ascontiguousarray(Wv[:, sl].astype(bfloat16)),
            "wo": np.ascontiguousarray(Wo[sl, :].astype(bfloat16)),
            "bq": np.ascontiguousarray(np.asarray(bq, np.float32)[sl, None]),
            "bk": np.ascontiguousarray(np.asarray(bk, np.float32)[sl, None]),
            "bv": np.ascontiguousarray(
                np.asarray(bv, np.float32)[None, sl].astype(bfloat16)),
        })

    res = run_bass_kernel_spmd(nc, in_maps, list(range(NCORES)), trace=trace)
    _cache["last_results"] = res

    # Chunked RS layout: core r's out rows [c*128:(c+1)*128] hold tokens
    # [c//2 * 2048 + (c%2)*1024 + r*128 : +128] of the flat [4096, 1024].
    out = np.zeros((TOK, D), np.float32)
    for r in range(NCORES):
        o = np.asarray(res.results[r]["out"]).astype(np.float32)
        for c in range(NCHUNK):
            t0 = (c // 2) * 2048 + (c % 2) * 1024 + r * 128
            out[t0:t0 + 128] = o[c * 128:(c + 1) * 128]
    out = out + np.asarray(bo, np.float32)[None, :]
    return out.reshape(B, S, D)


# revision 7
# speedup vs baseline: 2.5592x; 1.1162x over previous
"""Multi-head attention (B=2, S=2048, D=1024, H=16) on 8 NeuronCores.

Sharding: Megatron tensor parallelism. Core r owns heads 2r, 2r+1
(a 128-wide slice of D). Wq/Wk/Wv column-parallel, Wo row-parallel,
chunked ReduceScatter(add) over tokens, host reassembles and adds bo.

All matmul operands are bf16 (fp32 PSUM accumulate). Host pre-casts
activations/weights to bf16 and pre-transposes x to feature-major.

Per-core layouts:
  xqT/xkT/xvT : [1024, 4096] bf16  feature-major activations
  x tiles     : [128, 2048] per (tensor, k-tile, batch) - 24 DMAs/batch
  qT/kT       : [128, 2048] per batch; rows 0:64 = head0 dk, 64:128 = head1
  v           : [128, 130] x16 per batch; cols = [v_h0 | 1 | v_h1 | 1]
                (ones columns make the PV matmul emit softmax sums)
  scores      : psum [128 sk, 1024] = [h0 block | h1 block]; the two score
                matmuls run CONCURRENTLY via PE row tiling (K=64: h0 in
                array rows 0-63, h1 in rows 64-127)
  exp         : one ACT instr per [128, 1024] psum tile -> pt bf16 sbuf
  PV          : psum [65, 512] per head accumulated over 16 sk tiles;
                row 64 = softmax sums
  normalize   : sums -> PE broadcast (ones x sums) -> reciprocal_approx_fast
                on [128,512] (all lanes) -> one tensor_mul; no single-lane
                reciprocals on the critical path
  attnT       : [128, 2048] per batch, normalized, dk-major
  out-proj    : partial [tok, 1024] bf16 -> DRAM, ReduceScatter per token
                chunk (1024/1024/1024/512/512), overlapped with compute

The emission is software-pipelined: each (sq, sk) attention iteration
also pops one deferred thunk (previous block's normalization/out-proj,
remaining projection blocks of this batch, or the next batch's
projections) so the PE and ACT engines never drain.
"""

import sys

sys.path.insert(0, "/opt/trn_rl_repo")

import numpy as np

B, S, D, H, DK = 2, 2048, 1024, 16, 64
NCORES = 8
TOK = B * S            # 4096
DKC = D // NCORES      # 128 = 2 heads per core
KT = D // 128          # 8 contraction tiles
SKT = S // 128         # 16 key tiles per batch
SQB = S // 512         # 4 query blocks per batch

# ReduceScatter chunks: (batch, [sq blocks], rows per core)
CHUNKS = [
    (0, (0, 1), 128),
    (0, (2, 3), 128),
    (1, (0, 1), 128),
    (1, (2,), 64),
    (1, (3,), 64),
]
CHUNK_OFF = [0, 128, 256, 384, 448]   # row offset of each chunk in out_ext

_cache = {}


def _build(collective=True):
    from contextlib import ExitStack

    from concourse import bacc
    import concourse.mybir as mybir
    import concourse.tile as tile

    f32 = mybir.dt.float32
    bf16 = mybir.dt.bfloat16
    Act = mybir.ActivationFunctionType

    nc = bacc.Bacc(
        "TRN2", target_bir_lowering=False, debug=False,
        enable_asserts=False, num_devices=NCORES,
    )

    xqT = nc.dram_tensor("xqT", [D, TOK], bf16, kind="ExternalInput").ap()
    xkT = nc.dram_tensor("xkT", [D, TOK], bf16, kind="ExternalInput").ap()
    xvT = nc.dram_tensor("xvT", [D, TOK], bf16, kind="ExternalInput").ap()
    wq = nc.dram_tensor("wq", [D, DKC], bf16, kind="ExternalInput").ap()
    wk = nc.dram_tensor("wk", [D, DKC], bf16, kind="ExternalInput").ap()
    wv = nc.dram_tensor("wv", [D, DKC], bf16, kind="ExternalInput").ap()
    wo = nc.dram_tensor("wo", [DKC, D], bf16, kind="ExternalInput").ap()
    bq = nc.dram_tensor("bq", [DKC, 1], f32, kind="ExternalInput").ap()
    bk = nc.dram_tensor("bk", [DKC, 1], f32, kind="ExternalInput").ap()
    bv = nc.dram_tensor("bv", [1, DKC], bf16, kind="ExternalInput").ap()
    out_ext = nc.dram_tensor("out", [512, D], bf16, kind="ExternalOutput").ap()

    with tile.TileContext(nc) as tc, ExitStack() as ctx, \
            nc.allow_low_precision("bf16 matmul operands, fp32 psum accumulate"):
        wpool = ctx.enter_context(tc.tile_pool(name="w", bufs=1))
        xpool = ctx.enter_context(tc.tile_pool(name="x", bufs=9))
        qkpool = ctx.enter_context(tc.tile_pool(name="qk", bufs=2))
        vpool = ctx.enter_context(tc.tile_pool(name="v", bufs=32))
        ptpool = ctx.enter_context(tc.tile_pool(name="pt", bufs=4))
        atpool = ctx.enter_context(tc.tile_pool(name="at", bufs=2))
        smpool = ctx.enter_context(tc.tile_pool(name="sm", bufs=2))
        opool = ctx.enter_context(tc.tile_pool(name="o", bufs=4))
        # PSUM budget (8 banks): sc 2x[128,1024]=4, acc0+acc1=2, gen 2x[128,512]=2
        ps_sc = ctx.enter_context(tc.tile_pool(name="pssc", bufs=2, space="PSUM"))
        ps_acc = ctx.enter_context(tc.tile_pool(name="psacc", bufs=1, space="PSUM"))
        ps_gen = ctx.enter_context(tc.tile_pool(name="psgen", bufs=2, space="PSUM"))
        dram = ctx.enter_context(tc.tile_pool(name="dram", bufs=1, space="DRAM"))

        # ---- constants / weights into SBUF ----
        wq_t, wk_t, wv_t = [], [], []
        for name, src, lst in (("wq", wq, wq_t), ("wk", wk, wk_t), ("wv", wv, wv_t)):
            for k in range(KT):
                t = wpool.tile([128, DKC], bf16, tag=f"{name}{k}")
                nc.sync.dma_start(t[:], src[k * 128:(k + 1) * 128, :])
                lst.append(t)
        wo_t = wpool.tile([DKC, D], bf16, tag="wo")
        nc.sync.dma_start(wo_t[:], wo[:])
        bq_t = wpool.tile([DKC, 1], f32, tag="bq")
        nc.sync.dma_start(bq_t[:], bq[:])
        bk_t = wpool.tile([DKC, 1], f32, tag="bk")
        nc.sync.dma_start(bk_t[:], bk[:])
        bv_t = wpool.tile([1, DKC], bf16, tag="bv")
        nc.sync.dma_start(bv_t[:], bv[:])
        ones_t = wpool.tile([1, 128], bf16, tag="ones")
        nc.vector.memset(ones_t[:], 1.0)

        partials = [dram.tile([len(sqs) * 512, D], bf16, tag=f"partial{c}",
                              name=f"partial{c}")
                    for c, (_, sqs, _) in enumerate(CHUNKS)]
        rs_outs = [dram.tile([rows, D], bf16, tag=f"rsout{c}",
                             name=f"rsout{c}")
                   for c, (_, _, rows) in enumerate(CHUNKS)]

        # ---------- emission helpers ----------

        def load_x_batch(xT, b, tag):
            """DMA the 8 k-tiles [128, 2048] of one activation, one batch."""
            tiles = []
            for k in range(KT):
                xt = xpool.tile([128, S], bf16, tag=tag, name=f"{tag}{b}_{k}")
                nc.sync.dma_start(
                    xt[:], xT[k * 128:(k + 1) * 128, b * S:(b + 1) * S])
                tiles.append(xt)
            return tiles

        def qk_block(xts, w_list, bias_t, dst, blk):
            """One 512-col block of a q/k projection -> dst[:, blk]."""
            sl = slice(blk * 512, (blk + 1) * 512)
            ps = ps_gen.tile([128, 512], f32, tag="gen", name="psqk")
            for k in range(KT):
                nc.tensor.matmul(
                    ps[:], lhsT=w_list[k][:], rhs=xts[k][:, sl],
                    start=(k == 0), stop=(k == KT - 1),
                )
            nc.vector.tensor_scalar_add(dst[:, sl], ps[:], bias_t[:, 0:1])

        def v_block(xts, blk, v_tiles):
            """Four token-tiles [128, 130] of the v projection."""
            for mi in range(4):
                c0 = blk * 512 + mi * 128
                ps = ps_gen.tile([128, 512], f32, tag="gen", name="psv")
                for k in range(KT):
                    nc.tensor.matmul(
                        ps[:, 0:128], lhsT=xts[k][:, c0:c0 + 128],
                        rhs=wv_t[k][:], start=(k == 0), stop=False,
                    )
                nc.tensor.matmul(
                    ps[:, 0:128], lhsT=ones_t[0:1, :], rhs=bv_t[:],
                    start=False, stop=True,
                )
                vt = vpool.tile([128, 130], bf16, tag="v")
                nc.vector.tensor_copy(vt[:, 0:64], ps[:, 0:64])
                nc.vector.tensor_copy(vt[:, 65:129], ps[:, 64:128])
                nc.vector.memset(vt[:, 64:65], 1.0)
                nc.vector.memset(vt[:, 129:130], 1.0)
                v_tiles.append(vt)

        def emit_attn_iter(qT_b, kT_b, v_tiles, accs, sq, sk):
            """One (sq, sk) step: packed scores pair, one exp, two PV MMs."""
            qs = slice(sq * 512, (sq + 1) * 512)
            ks = slice(sk * 128, (sk + 1) * 128)
            sps = ps_sc.tile([128, 1024], f32, tag="sc")
            # two heads run concurrently in PE rows 0-63 / 64-127
            nc.tensor.matmul(
                sps[:, 0:512], lhsT=kT_b[0:64, ks], rhs=qT_b[0:64, qs],
                start=True, stop=True,
            )
            nc.tensor.matmul(
                sps[:, 512:1024], lhsT=kT_b[64:128, ks], rhs=qT_b[64:128, qs],
                start=True, stop=True,
            )
            pt = ptpool.tile([128, 1024], bf16, tag="pt")
            nc.scalar.activation(pt[:], sps[:], Act.Exp, scale=0.125)
            nc.tensor.matmul(
                accs[0][:], lhsT=v_tiles[sk][:, 0:65], rhs=pt[:, 0:512],
                start=(sk == 0), stop=(sk == SKT - 1),
            )
            nc.tensor.matmul(
                accs[1][:], lhsT=v_tiles[sk][:, 65:130], rhs=pt[:, 512:1024],
                start=(sk == 0), stop=(sk == SKT - 1),
            )

        def drain_accs(accs):
            """Inline epilogue part 1 (cheap DVE copies only): pull sums and
            raw attention out of PSUM so the acc banks free up fast."""
            sums = smpool.tile([1, 1024], bf16, tag="sums")
            araw = smpool.tile([128, 512], bf16, tag="araw")
            nc.vector.tensor_copy(sums[0:1, 0:512], accs[0][64:65, :])
            nc.vector.tensor_copy(sums[0:1, 512:1024], accs[1][64:65, :])
            nc.vector.tensor_copy(araw[0:64, :], accs[0][0:64, :])
            nc.vector.tensor_copy(araw[64:128, :], accs[1][0:64, :])
            return sums, araw

        def make_epilogue(attnT_b, sums, araw, b, sq):
            """Deferred epilogue thunks for (b, sq): broadcast-normalize,
            out-projection, and (if chunk-final) the chunk's ReduceScatter."""
            qs = slice(sq * 512, (sq + 1) * 512)

            def s_norm():
                rbs_ps = ps_gen.tile([128, 512], f32, tag="gen", name="rbs_ps")
                # the two broadcast matmuls run concurrently (col tiling)
                nc.tensor.matmul(
                    rbs_ps[0:64, :], lhsT=ones_t[0:1, 0:64],
                    rhs=sums[0:1, 0:512], start=True, stop=True,
                )
                nc.tensor.matmul(
                    rbs_ps[64:128, :], lhsT=ones_t[0:1, 0:64],
                    rhs=sums[0:1, 512:1024], start=True, stop=True,
                )
                rb = smpool.tile([128, 512], f32, tag="rb")
                nc.vector.reciprocal_approx_fast(rb[:], rbs_ps[:])
                nc.vector.tensor_mul(attnT_b[:, qs], araw[:], rb[:])

            def out_m(m):
                def f():
                    col = sq * 512 + m * 128
                    chunk = next(c for c, (bb, sqs, _) in enumerate(CHUNKS)
                                 if bb == b and sq in sqs)
                    srow = CHUNKS[chunk][1].index(sq) * 512 + m * 128
                    for n2 in range(2):
                        ops = ps_gen.tile([128, 512], f32, tag="gen",
                                          name="psout")
                        nc.tensor.matmul(
                            ops[:], lhsT=attnT_b[:, col:col + 128],
                            rhs=wo_t[:, n2 * 512:(n2 + 1) * 512],
                            start=True, stop=True,
                        )
                        ot = opool.tile([128, 512], bf16, tag="ot")
                        nc.vector.tensor_copy(ot[:], ops[:])
                        nc.sync.dma_start(
                            partials[chunk][srow:srow + 128,
                                            n2 * 512:(n2 + 1) * 512],
                            ot[:],
                        )
                return f

            thunks = [s_norm] + [out_m(m) for m in range(4)]
            chunk = next(c for c, (bb, sqs, _) in enumerate(CHUNKS)
                         if bb == b and sq == sqs[-1])\
                if any(bb == b and sq == sqs[-1] for bb, sqs, _ in CHUNKS) else None
            if chunk is not None and collective:
                rows = CHUNKS[chunk][2]
                off = CHUNK_OFF[chunk]

                def rs():
                    nc.gpsimd.collective_compute(
                        "ReduceScatter",
                        mybir.AluOpType.add,
                        replica_groups=[list(range(NCORES))],
                        ins=[partials[chunk].opt()],
                        outs=[rs_outs[chunk].opt()],
                    )
                    nc.sync.dma_start(
                        out_ext[off:off + rows, :], rs_outs[chunk][:])
                thunks.append(rs)
            return thunks

        # ---------- main emission ----------
        qT, kT, attnT = {}, {}, {}
        vt_all = {0: [], 1: []}
        for b in (0, 1):
            kT[b] = qkpool.tile([128, S], bf16, tag="kT", name=f"kT{b}")
            qT[b] = qkpool.tile([128, S], bf16, tag="qT", name=f"qT{b}")
            attnT[b] = atpool.tile([128, S], bf16, tag="attnT",
                                   name=f"attnT{b}")

        # batch 0: DMAs + the minimal prefix needed to start attention
        xk0 = load_x_batch(xkT, 0, "xk")
        xq0 = load_x_batch(xqT, 0, "xq")
        xv0 = load_x_batch(xvT, 0, "xv")
        qk_block(xk0, wk_t, bk_t, kT[0], 0)
        qk_block(xq0, wq_t, bq_t, qT[0], 0)
        v_block(xv0, 0, vt_all[0])

        # the rest of b0's projections + all of b1's become deferred thunks
        extra = []
        for blk in (1, 2, 3):
            extra.append(lambda blk=blk: qk_block(xk0, wk_t, bk_t, kT[0], blk))
        extra.append(lambda: v_block(xv0, 1, vt_all[0]))
        extra.append(lambda: qk_block(xq0, wq_t, bq_t, qT[0], 1))
        extra.append(lambda: v_block(xv0, 2, vt_all[0]))
        extra.append(lambda: qk_block(xq0, wq_t, bq_t, qT[0], 2))
        extra.append(lambda: v_block(xv0, 3, vt_all[0]))
        extra.append(lambda: qk_block(xq0, wq_t, bq_t, qT[0], 3))

        xb1 = {}

        def b1_dma(name, xT):
            def f():
                xb1[name] = load_x_batch(xT, 1, name)
            return f

        extra.append(b1_dma("xk", xkT))
        for blk in range(SQB):
            extra.append(lambda blk=blk: qk_block(
                xb1["xk"], wk_t, bk_t, kT[1], blk))
        extra.append(b1_dma("xq", xqT))
        for blk in range(SQB):
            extra.append(lambda blk=blk: qk_block(
                xb1["xq"], wq_t, bq_t, qT[1], blk))
        extra.append(b1_dma("xv", xvT))
        for blk in range(SQB):
            extra.append(lambda blk=blk: v_block(xb1["xv"], blk, vt_all[1]))

        pending = []

        def emit_batch_attention(b):
            for sq in range(SQB):
                accs = (
                    ps_acc.tile([65, 512], f32, tag="acc0", name="acc0"),
                    ps_acc.tile([65, 512], f32, tag="acc1", name="acc1"),
                )
                for sk in range(SKT):
                    emit_attn_iter(qT[b], kT[b], vt_all[b], accs, sq, sk)
                    if pending:
                        pending.pop(0)()
                    elif extra:
                        extra.pop(0)()
                sums, araw = drain_accs(accs)
                pending.extend(make_epilogue(attnT[b], sums, araw, b, sq))

        emit_batch_attention(0)
        # all of b1's projection thunks must be emitted before b1's
        # attention reads their outputs (deps are tracked in program order)
        while extra:
            extra.pop(0)()
        emit_batch_attention(1)
        for f in pending:
            f()
        pending.clear()
        while extra:
            extra.pop(0)()

        if not collective:
            for c, (_, _, rows) in enumerate(CHUNKS):
                nc.sync.dma_start(
                    out_ext[CHUNK_OFF[c]:CHUNK_OFF[c] + rows, :],
                    partials[c][0:rows, :],
                )

    nc.compile()
    return nc


def _get_nc():
    if "nc" not in _cache:
        _cache["nc"] = _build()
    return _cache["nc"]


def kernel(query, key, value, Wq, bq, Wk, bk, Wv, bv, Wo, bo, trace=False):
    from concourse.bass_utils import run_bass_kernel_spmd
    import ml_dtypes

    bfloat16 = ml_dtypes.bfloat16
    nc = _get_nc()

    q = np.ascontiguousarray(
        np.asarray(query, np.float32).reshape(TOK, D).T.astype(bfloat16))
    k = np.ascontiguousarray(
        np.asarray(key, np.float32).reshape(TOK, D).T.astype(bfloat16))
    v = np.ascontiguousarray(
        np.asarray(value, np.float32).reshape(TOK, D).T.astype(bfloat16))
    Wq = np.asarray(Wq, np.float32)
    Wk = np.asarray(Wk, np.float32)
    Wv = np.asarray(Wv, np.float32)
    Wo = np.asarray(Wo, np.float32)

    in_maps = []
    for r in range(NCORES):
        sl = slice(r * DKC, (r + 1) * DKC)
        in_maps.append({
            "xqT": q, "xkT": k, "xvT": v,
            "wq": np.ascontiguousarray(Wq[:, sl].astype(bfloat16)),
            "wk": np.ascontiguousarray(Wk[:, sl].astype(bfloat16)),
            "wv": np.ascontiguousarray(Wv[:, sl].astype(bfloat16)),
            "wo": np.ascontiguousarray(Wo[sl, :].astype(bfloat16)),
            "bq": np.ascontiguousarray(np.asarray(bq, np.float32)[sl, None]),
            "bk": np.ascontiguousarray(np.asarray(bk, np.float32)[sl, None]),
            "bv": np.ascontiguousarray(
                np.asarray(bv, np.float32)[None, sl].astype(bfloat16)),
        })

    res = run_bass_kernel_spmd(nc, in_maps, list(range(NCORES)), trace=trace)
    _cache["last_results"] = res

    # Reassemble: chunk c scatters its rows over cores; core r's piece of
    # chunk c sits at out_ext[CHUNK_OFF[c] : +rows].
    out = np.zeros((TOK, D), np.float32)
    for r in range(NCORES):
        o = np.asarray(res.results[r]["out"]).astype(np.float32)
        for c, (b, sqs, rows) in enumerate(CHUNKS):
            t0 = b * S + sqs[0] * 512 + r * rows
            off = CHUNK_OFF[c]
            out[t0:t0 + rows] = o[off:off + rows]
    out = out + np.asarray(bo, np.float32)[None, :]
    return out.reshape(B, S, D)


# revision 8
# speedup vs baseline: 2.6897x; 1.0510x over previous
"""Multi-head attention (B=2, S=2048, D=1024, H=16) on 8 NeuronCores.

Sharding: Megatron tensor parallelism. Core r owns heads 2r, 2r+1
(a 128-wide slice of D). Wq/Wk/Wv column-parallel, Wo row-parallel,
chunked ReduceScatter(add) over tokens, host reassembles and adds bo.

All matmul operands are bf16 (fp32 PSUM accumulate). Host pre-casts
activations/weights to bf16 and pre-transposes x to feature-major.

Per-core layouts:
  xqT/xkT/xvT : [1024, 4096] bf16  feature-major activations
  x tiles     : [128, 2048] per (tensor, k-tile, batch) - 24 DMAs/batch
  qT/kT       : [128, 2048] per batch; rows 0:64 = head0 dk, 64:128 = head1
  v           : [128, 130] x16 per batch; cols = [v_h0 | 1 | v_h1 | 1]
                (ones columns make the PV matmul emit softmax sums)
  scores      : psum [128 sk, 1024] = [h0 block | h1 block]; the two score
                matmuls run CONCURRENTLY via PE row tiling (K=64: h0 in
                array rows 0-63, h1 in rows 64-127)
  exp         : one ACT instr per [128, 1024] psum tile -> pt bf16 sbuf
  PV          : psum [65, 512] per head accumulated over 16 sk tiles;
                row 64 = softmax sums
  normalize   : sums -> PE broadcast (ones x sums) -> reciprocal_approx_fast
                on [128,512] (all lanes) -> one tensor_mul; no single-lane
                reciprocals on the critical path
  attnT       : [128, 2048] per batch, normalized, dk-major
  out-proj    : partial [tok, 1024] bf16 -> DRAM, ReduceScatter per token
                chunk (1024/1024/1024/512/512), overlapped with compute

The emission is software-pipelined: each (sq, sk) attention iteration
also pops one deferred thunk (previous block's normalization/out-proj,
remaining projection blocks of this batch, or the next batch's
projections) so the PE and ACT engines never drain.
"""

import sys

sys.path.insert(0, "/opt/trn_rl_repo")

import numpy as np

B, S, D, H, DK = 2, 2048, 1024, 16, 64
NCORES = 8
TOK = B * S            # 4096
DKC = D // NCORES      # 128 = 2 heads per core
KT = D // 128          # 8 contraction tiles
SKT = S // 128         # 16 key tiles per batch
SQB = S // 512         # 4 query blocks per batch

# ReduceScatter chunks: (batch, [sq blocks], rows per core)
CHUNKS = [
    (0, (0, 1), 128),
    (0, (2, 3), 128),
    (1, (0, 1), 128),
    (1, (2,), 64),
    (1, (3,), 64),
]
CHUNK_OFF = [0, 128, 256, 384, 448]   # row offset of each chunk in out_ext

_cache = {}


def _build(collective=True):
    from contextlib import ExitStack

    from concourse import bacc
    import concourse.mybir as mybir
    import concourse.tile as tile

    f32 = mybir.dt.float32
    bf16 = mybir.dt.bfloat16
    Act = mybir.ActivationFunctionType

    nc = bacc.Bacc(
        "TRN2", target_bir_lowering=False, debug=False,
        enable_asserts=False, num_devices=NCORES,
    )

    xqT = nc.dram_tensor("xqT", [D, TOK], bf16, kind="ExternalInput").ap()
    xkT = nc.dram_tensor("xkT", [D, TOK], bf16, kind="ExternalInput").ap()
    xvT = nc.dram_tensor("xvT", [D, TOK], bf16, kind="ExternalInput").ap()
    wq = nc.dram_tensor("wq", [D, DKC], bf16, kind="ExternalInput").ap()
    wk = nc.dram_tensor("wk", [D, DKC], bf16, kind="ExternalInput").ap()
    wv = nc.dram_tensor("wv", [D, DKC], bf16, kind="ExternalInput").ap()
    wo = nc.dram_tensor("wo", [DKC, D], bf16, kind="ExternalInput").ap()
    bq = nc.dram_tensor("bq", [DKC, 1], f32, kind="ExternalInput").ap()
    bk = nc.dram_tensor("bk", [DKC, 1], f32, kind="ExternalInput").ap()
    bv = nc.dram_tensor("bv", [1, DKC], bf16, kind="ExternalInput").ap()
    out_ext = nc.dram_tensor("out", [512, D], bf16, kind="ExternalOutput").ap()

    with tile.TileContext(nc) as tc, ExitStack() as ctx, \
            nc.allow_low_precision("bf16 matmul operands, fp32 psum accumulate"):
        wpool = ctx.enter_context(tc.tile_pool(name="w", bufs=1))
        xpool = ctx.enter_context(tc.tile_pool(name="x", bufs=18))
        qkpool = ctx.enter_context(tc.tile_pool(name="qk", bufs=2))
        vpool = ctx.enter_context(tc.tile_pool(name="v", bufs=32))
        ptpool = ctx.enter_context(tc.tile_pool(name="pt", bufs=6))
        atpool = ctx.enter_context(tc.tile_pool(name="at", bufs=2))
        smpool = ctx.enter_context(tc.tile_pool(name="sm", bufs=2))
        opool = ctx.enter_context(tc.tile_pool(name="o", bufs=4))
        # PSUM budget (8 banks): sc 2x[128,1024]=4, acc0+acc1=2, gen 2x[128,512]=2
        ps_sc = ctx.enter_context(tc.tile_pool(name="pssc", bufs=2, space="PSUM"))
        ps_acc = ctx.enter_context(tc.tile_pool(name="psacc", bufs=1, space="PSUM"))
        ps_gen = ctx.enter_context(tc.tile_pool(name="psgen", bufs=2, space="PSUM"))
        dram = ctx.enter_context(tc.tile_pool(name="dram", bufs=1, space="DRAM"))

        # ---- constants / weights into SBUF ----
        wq_t, wk_t, wv_t = [], [], []
        for name, src, lst in (("wq", wq, wq_t), ("wk", wk, wk_t), ("wv", wv, wv_t)):
            for k in range(KT):
                t = wpool.tile([128, DKC], bf16, tag=f"{name}{k}")
                nc.sync.dma_start(t[:], src[k * 128:(k + 1) * 128, :])
                lst.append(t)
        wo_t = wpool.tile([DKC, D], bf16, tag="wo")
        nc.sync.dma_start(wo_t[:], wo[:])
        bq_t = wpool.tile([DKC, 1], f32, tag="bq")
        nc.sync.dma_start(bq_t[:], bq[:])
        bk_t = wpool.tile([DKC, 1], f32, tag="bk")
        nc.sync.dma_start(bk_t[:], bk[:])
        bv_t = wpool.tile([1, DKC], bf16, tag="bv")
        nc.sync.dma_start(bv_t[:], bv[:])
        ones_t = wpool.tile([1, 128], bf16, tag="ones")
        nc.vector.memset(ones_t[:], 1.0)

        partials = [dram.tile([len(sqs) * 512, D], bf16, tag=f"partial{c}",
                              name=f"partial{c}")
                    for c, (_, sqs, _) in enumerate(CHUNKS)]
        rs_outs = [dram.tile([rows, D], bf16, tag=f"rsout{c}",
                             name=f"rsout{c}")
                   for c, (_, _, rows) in enumerate(CHUNKS)]

        # ---------- emission helpers ----------

        def load_x_half(xT, b, h, tag):
            """DMA the 8 k-tiles [128, 1024] of one activation half-batch."""
            tiles = []
            for k in range(KT):
                xt = xpool.tile([128, 1024], bf16, tag=tag,
                                name=f"{tag}{b}_{h}_{k}")
                c0 = b * S + h * 1024
                nc.sync.dma_start(
                    xt[:], xT[k * 128:(k + 1) * 128, c0:c0 + 1024])
                tiles.append(xt)
            return tiles

        def qk_block(xts_halves, w_list, bias_t, dst, blk):
            """One 512-col block of a q/k projection -> dst[:, blk]."""
            xts = xts_halves[blk // 2]
            hsl = slice((blk % 2) * 512, (blk % 2 + 1) * 512)
            ps = ps_gen.tile([128, 512], f32, tag="gen", name="psqk")
            for k in range(KT):
                nc.tensor.matmul(
                    ps[:], lhsT=w_list[k][:], rhs=xts[k][:, hsl],
                    start=(k == 0), stop=(k == KT - 1),
                )
            nc.vector.tensor_scalar_add(
                dst[:, blk * 512:(blk + 1) * 512], ps[:], bias_t[:, 0:1])

        def v_block(xts_halves, blk, v_tiles):
            """Four token-tiles [128, 130] of the v projection."""
            xts = xts_halves[blk // 2]
            for mi in range(4):
                c0 = (blk % 2) * 512 + mi * 128
                ps = ps_gen.tile([128, 512], f32, tag="gen", name="psv")
                for k in range(KT):
                    nc.tensor.matmul(
                        ps[:, 0:128], lhsT=xts[k][:, c0:c0 + 128],
                        rhs=wv_t[k][:], start=(k == 0), stop=False,
                    )
                nc.tensor.matmul(
                    ps[:, 0:128], lhsT=ones_t[0:1, :], rhs=bv_t[:],
                    start=False, stop=True,
                )
                vt = vpool.tile([128, 130], bf16, tag="v")
                nc.vector.tensor_copy(vt[:, 0:64], ps[:, 0:64])
                nc.vector.tensor_copy(vt[:, 65:129], ps[:, 64:128])
                nc.vector.memset(vt[:, 64:65], 1.0)
                nc.vector.memset(vt[:, 129:130], 1.0)
                v_tiles.append(vt)

        def emit_attn_iter(qT_b, kT_b, v_tiles, accs, sq, sk):
            """One (sq, sk) step: packed scores pair, one exp, two PV MMs."""
            qs = slice(sq * 512, (sq + 1) * 512)
            ks = slice(sk * 128, (sk + 1) * 128)
            sps = ps_sc.tile([128, 1024], f32, tag="sc")
            # two heads run concurrently in PE rows 0-63 / 64-127
            nc.tensor.matmul(
                sps[:, 0:512], lhsT=kT_b[0:64, ks], rhs=qT_b[0:64, qs],
                start=True, stop=True,
            )
            nc.tensor.matmul(
                sps[:, 512:1024], lhsT=kT_b[64:128, ks], rhs=qT_b[64:128, qs],
                start=True, stop=True,
            )
            pt = ptpool.tile([128, 1024], bf16, tag="pt")
            nc.scalar.activation(pt[:], sps[:], Act.Exp, scale=0.125)
            nc.tensor.matmul(
                accs[0][:], lhsT=v_tiles[sk][:, 0:65], rhs=pt[:, 0:512],
                start=(sk == 0), stop=(sk == SKT - 1),
            )
            nc.tensor.matmul(
                accs[1][:], lhsT=v_tiles[sk][:, 65:130], rhs=pt[:, 512:1024],
                start=(sk == 0), stop=(sk == SKT - 1),
            )

        def drain_accs(accs):
            """Inline epilogue part 1 (cheap DVE copies only): pull sums and
            raw attention out of PSUM so the acc banks free up fast."""
            sums = smpool.tile([1, 1024], bf16, tag="sums")
            araw = smpool.tile([128, 512], bf16, tag="araw")
            nc.vector.tensor_copy(sums[0:1, 0:512], accs[0][64:65, :])
            nc.vector.tensor_copy(sums[0:1, 512:1024], accs[1][64:65, :])
            nc.vector.tensor_copy(araw[0:64, :], accs[0][0:64, :])
            nc.vector.tensor_copy(araw[64:128, :], accs[1][0:64, :])
            return sums, araw

        def make_epilogue(attnT_b, sums, araw, b, sq):
            """Deferred epilogue thunks for (b, sq): broadcast-normalize,
            out-projection, and (if chunk-final) the chunk's ReduceScatter."""
            qs = slice(sq * 512, (sq + 1) * 512)

            def s_norm():
                rbs_ps = ps_gen.tile([128, 512], f32, tag="gen", name="rbs_ps")
                # the two broadcast matmuls run concurrently (col tiling)
                nc.tensor.matmul(
                    rbs_ps[0:64, :], lhsT=ones_t[0:1, 0:64],
                    rhs=sums[0:1, 0:512], start=True, stop=True,
                )
                nc.tensor.matmul(
                    rbs_ps[64:128, :], lhsT=ones_t[0:1, 0:64],
                    rhs=sums[0:1, 512:1024], start=True, stop=True,
                )
                rb = smpool.tile([128, 512], f32, tag="rb")
                nc.vector.reciprocal_approx_fast(rb[:], rbs_ps[:])
                nc.vector.tensor_mul(attnT_b[:, qs], araw[:], rb[:])

            def out_m(m):
                def f():
                    col = sq * 512 + m * 128
                    chunk = next(c for c, (bb, sqs, _) in enumerate(CHUNKS)
                                 if bb == b and sq in sqs)
                    srow = CHUNKS[chunk][1].index(sq) * 512 + m * 128
                    for n2 in range(2):
                        ops = ps_gen.tile([128, 512], f32, tag="gen",
                                          name="psout")
                        nc.tensor.matmul(
                            ops[:], lhsT=attnT_b[:, col:col + 128],
                            rhs=wo_t[:, n2 * 512:(n2 + 1) * 512],
                            start=True, stop=True,
                        )
                        ot = opool.tile([128, 512], bf16, tag="ot")
                        nc.vector.tensor_copy(ot[:], ops[:])
                        nc.sync.dma_start(
                            partials[chunk][srow:srow + 128,
                                            n2 * 512:(n2 + 1) * 512],
                            ot[:],
                        )
                return f

            thunks = [s_norm] + [out_m(m) for m in range(4)]
            chunk = next(c for c, (bb, sqs, _) in enumerate(CHUNKS)
                         if bb == b and sq == sqs[-1])\
                if any(bb == b and sq == sqs[-1] for bb, sqs, _ in CHUNKS) else None
            if chunk is not None and collective:
                rows = CHUNKS[chunk][2]
                off = CHUNK_OFF[chunk]

                def rs():
                    nc.gpsimd.collective_compute(
                        "ReduceScatter",
                        mybir.AluOpType.add,
                        replica_groups=[list(range(NCORES))],
                        ins=[partials[chunk].opt()],
                        outs=[rs_outs[chunk].opt()],
                    )
                    nc.sync.dma_start(
                        out_ext[off:off + rows, :], rs_outs[chunk][:])
                thunks.append(rs)
            return thunks

        # ---------- main emission ----------
        qT, kT, attnT = {}, {}, {}
        vt_all = {0: [], 1: []}
        for b in (0, 1):
            kT[b] = qkpool.tile([128, S], bf16, tag="kT", name=f"kT{b}")
            qT[b] = qkpool.tile([128, S], bf16, tag="qT", name=f"qT{b}")
            attnT[b] = atpool.tile([128, S], bf16, tag="attnT",
                                   name=f"attnT{b}")

        # batch 0: first-half x DMAs land before anything else so the
        # first projection blocks can start ~immediately
        xb0 = {"xk": [None, None], "xq": [None, None], "xv": [None, None]}
        xb1 = {"xk": [None, None], "xq": [None, None], "xv": [None, None]}
        xb0["xk"][0] = load_x_half(xkT, 0, 0, "xk")
        xb0["xq"][0] = load_x_half(xqT, 0, 0, "xq")
        xb0["xv"][0] = load_x_half(xvT, 0, 0, "xv")
        xb0["xk"][1] = load_x_half(xkT, 0, 1, "xk")
        xb0["xq"][1] = load_x_half(xqT, 0, 1, "xq")
        xb0["xv"][1] = load_x_half(xvT, 0, 1, "xv")
        qk_block(xb0["xk"], wk_t, bk_t, kT[0], 0)
        qk_block(xb0["xq"], wq_t, bq_t, qT[0], 0)
        v_block(xb0["xv"], 0, vt_all[0])

        def b1_dma(name, xT, h):
            def f():
                xb1[name][h] = load_x_half(xT, 1, h, name)
            return f

        # the rest of b0's projections + all of b1's become deferred thunks
        extra = [
            lambda: qk_block(xb0["xk"], wk_t, bk_t, kT[0], 1),
            lambda: v_block(xb0["xv"], 1, vt_all[0]),
            lambda: qk_block(xb0["xq"], wq_t, bq_t, qT[0], 1),
            lambda: qk_block(xb0["xk"], wk_t, bk_t, kT[0], 2),
            b1_dma("xk", xkT, 0),
            lambda: v_block(xb0["xv"], 2, vt_all[0]),
            lambda: qk_block(xb0["xq"], wq_t, bq_t, qT[0], 2),
            lambda: qk_block(xb0["xk"], wk_t, bk_t, kT[0], 3),
            b1_dma("xk", xkT, 1),
            lambda: v_block(xb0["xv"], 3, vt_all[0]),
            lambda: qk_block(xb0["xq"], wq_t, bq_t, qT[0], 3),
            b1_dma("xq", xqT, 0),
            b1_dma("xv", xvT, 0),
            b1_dma("xq", xqT, 1),
            b1_dma("xv", xvT, 1),
        ]
        for blk in range(SQB):
            extra.append(lambda blk=blk: qk_block(
                xb1["xk"], wk_t, bk_t, kT[1], blk))
        for blk in range(SQB):
            extra.append(lambda blk=blk: qk_block(
                xb1["xq"], wq_t, bq_t, qT[1], blk))
        for blk in range(SQB):
            extra.append(lambda blk=blk: v_block(xb1["xv"], blk, vt_all[1]))

        pending = []

        def emit_batch_attention(b):
            for sq in range(SQB):
                accs = (
                    ps_acc.tile([65, 512], f32, tag="acc0", name="acc0"),
                    ps_acc.tile([65, 512], f32, tag="acc1", name="acc1"),
                )
                for sk in range(SKT):
                    emit_attn_iter(qT[b], kT[b], vt_all[b], accs, sq, sk)
                    if sk >= SKT - 2:
                        continue  # keep the boundary clean for the drain
                    if pending:
                        pending.pop(0)()
                    elif extra:
                        extra.pop(0)()
                sums, araw = drain_accs(accs)
                pending.extend(make_epilogue(attnT[b], sums, araw, b, sq))

        emit_batch_attention(0)
        # all of b1's projection thunks must be emitted before b1's
        # attention reads their outputs (deps are tracked in program order)
        while extra:
            extra.pop(0)()
        emit_batch_attention(1)
        for f in pending:
            f()
        pending.clear()
        while extra:
            extra.pop(0)()

        if not collective:
            for c, (_, _, rows) in enumerate(CHUNKS):
                nc.sync.dma_start(
                    out_ext[CHUNK_OFF[c]:CHUNK_OFF[c] + rows, :],
                    partials[c][0:rows, :],
                )

    nc.compile()
    return nc


def _get_nc():
    if "nc" not in _cache:
        _cache["nc"] = _build()
    return _cache["nc"]


def kernel(query, key, value, Wq, bq, Wk, bk, Wv, bv, Wo, bo, trace=False):
    from concourse.bass_utils import run_bass_kernel_spmd
    import ml_dtypes

    bfloat16 = ml_dtypes.bfloat16
    nc = _get_nc()

    q = np.ascontiguousarray(
        np.asarray(query, np.float32).reshape(TOK, D).T.astype(bfloat16))
    k = np.ascontiguousarray(
        np.asarray(key, np.float32).reshape(TOK, D).T.astype(bfloat16))
    v = np.ascontiguousarray(
        np.asarray(value, np.float32).reshape(TOK, D).T.astype(bfloat16))
    Wq = np.asarray(Wq, np.float32)
    Wk = np.asarray(Wk, np.float32)
    Wv = np.asarray(Wv, np.float32)
    Wo = np.asarray(Wo, np.float32)

    in_maps = []
    for r in range(NCORES):
        sl = slice(r * DKC, (r + 1) * DKC)
        in_maps.append({
            "xqT": q, "xkT": k, "xvT": v,
            "wq": np.ascontiguousarray(Wq[:, sl].astype(bfloat16)),
            "wk": np.ascontiguousarray(Wk[:, sl].astype(bfloat16)),
            "wv": np.ascontiguousarray(Wv[:, sl].astype(bfloat16)),
            "wo": np.ascontiguousarray(Wo[sl, :].astype(bfloat16)),
            "bq": np.ascontiguousarray(np.asarray(bq, np.float32)[sl, None]),
            "bk": np.ascontiguousarray(np.asarray(bk, np.float32)[sl, None]),
            "bv": np.ascontiguousarray(
                np.asarray(bv, np.float32)[None, sl].astype(bfloat16)),
        })

    res = run_bass_kernel_spmd(nc, in_maps, list(range(NCORES)), trace=trace)
    _cache["last_results"] = res

    # Reassemble: chunk c scatters its rows over cores; core r's piece of
    # chunk c sits at out_ext[CHUNK_OFF[c] : +rows].
    out = np.zeros((TOK, D), np.float32)
    for r in range(NCORES):
        o = np.asarray(res.results[r]["out"]).astype(np.float32)
        for c, (b, sqs, rows) in enumerate(CHUNKS):
            t0 = b * S + sqs[0] * 512 + r * rows
            off = CHUNK_OFF[c]
            out[t0:t0 + rows] = o[off:off + rows]
    out = out + np.asarray(bo, np.float32)[None, :]
    return out.reshape(B, S, D)


# revision 10
# speedup vs baseline: 2.7038x; 1.0052x over previous
"""Multi-head attention (B=2, S=2048, D=1024, H=16) on 8 NeuronCores.

Sharding: Megatron tensor parallelism. Core r owns heads 2r, 2r+1
(a 128-wide slice of D). Wq/Wk/Wv column-parallel, Wo row-parallel,
chunked ReduceScatter(add) over tokens, host reassembles and adds bo.

All matmul operands are bf16 (fp32 PSUM accumulate). Host pre-casts
activations/weights to bf16 and pre-transposes x to feature-major.

Per-core layouts:
  xqT/xkT/xvT : [1024, 4096] bf16  feature-major activations
  x tiles     : [128, 2048] per (tensor, k-tile, batch) - 24 DMAs/batch
  qT/kT       : [128, 2048] per batch; rows 0:64 = head0 dk, 64:128 = head1
  v           : [128, 130] x16 per batch; cols = [v_h0 | 1 | v_h1 | 1]
                (ones columns make the PV matmul emit softmax sums)
  scores      : psum [128 sk, 1024] = [h0 block | h1 block]; the two score
                matmuls run CONCURRENTLY via PE row tiling (K=64: h0 in
                array rows 0-63, h1 in rows 64-127)
  exp         : one ACT instr per [128, 1024] psum tile -> pt bf16 sbuf
  PV          : psum [65, 512] per head accumulated over 16 sk tiles;
                row 64 = softmax sums
  normalize   : sums -> PE broadcast (ones x sums) -> reciprocal_approx_fast
                on [128,512] (all lanes) -> one tensor_mul; no single-lane
                reciprocals on the critical path
  attnT       : [128, 2048] per batch, normalized, dk-major
  out-proj    : partial [tok, 1024] bf16 -> DRAM, ReduceScatter per token
                chunk (1024/1024/1024/512/512), overlapped with compute

The emission is software-pipelined: each (sq, sk) attention iteration
also pops one deferred thunk (previous block's normalization/out-proj,
remaining projection blocks of this batch, or the next batch's
projections) so the PE and ACT engines never drain.
"""

import sys

sys.path.insert(0, "/opt/trn_rl_repo")

import numpy as np

B, S, D, H, DK = 2, 2048, 1024, 16, 64
NCORES = 8
TOK = B * S            # 4096
DKC = D // NCORES      # 128 = 2 heads per core
KT = D // 128          # 8 contraction tiles
SKT = S // 128         # 16 key tiles per batch
SQB = S // 512         # 4 query blocks per batch

# ReduceScatter chunks: (batch, [sq blocks], rows per core)
CHUNKS = [
    (0, (0, 1), 128),
    (0, (2, 3), 128),
    (1, (0, 1), 128),
    (1, (2,), 64),
    (1, (3,), 64),
]
CHUNK_OFF = [0, 128, 256, 384, 448]   # row offset of each chunk in out_ext

_cache = {}


def _build(collective=True):
    from contextlib import ExitStack

    from concourse import bacc
    import concourse.mybir as mybir
    import concourse.tile as tile

    f32 = mybir.dt.float32
    bf16 = mybir.dt.bfloat16
    Act = mybir.ActivationFunctionType

    nc = bacc.Bacc(
        "TRN2", target_bir_lowering=False, debug=False,
        enable_asserts=False, num_devices=NCORES,
    )

    # x tensors host-arranged as [4, 128, 8192]: row block b*2+h holds
    # tokens [b*2048+h*1024 : +1024), cols = k-tile * 1024 + token offset
    xqT = nc.dram_tensor("xqT", [512, 8192], bf16, kind="ExternalInput").ap()
    xkT = nc.dram_tensor("xkT", [512, 8192], bf16, kind="ExternalInput").ap()
    xvT = nc.dram_tensor("xvT", [512, 8192], bf16, kind="ExternalInput").ap()
    # weights host-arranged as [128, 8*128]: k-tiles side by side
    wq = nc.dram_tensor("wq", [128, D], bf16, kind="ExternalInput").ap()
    wk = nc.dram_tensor("wk", [128, D], bf16, kind="ExternalInput").ap()
    wv = nc.dram_tensor("wv", [128, D], bf16, kind="ExternalInput").ap()
    wo = nc.dram_tensor("wo", [DKC, D], bf16, kind="ExternalInput").ap()
    bq = nc.dram_tensor("bq", [DKC, 1], f32, kind="ExternalInput").ap()
    bk = nc.dram_tensor("bk", [DKC, 1], f32, kind="ExternalInput").ap()
    bv = nc.dram_tensor("bv", [1, DKC], bf16, kind="ExternalInput").ap()
    out_ext = nc.dram_tensor("out", [512, D], bf16, kind="ExternalOutput").ap()

    with tile.TileContext(nc) as tc, ExitStack() as ctx, \
            nc.allow_low_precision("bf16 matmul operands, fp32 psum accumulate"):
        wpool = ctx.enter_context(tc.tile_pool(name="w", bufs=1))
        xpool = ctx.enter_context(tc.tile_pool(name="x", bufs=2))
        qkpool = ctx.enter_context(tc.tile_pool(name="qk", bufs=2))
        vpool = ctx.enter_context(tc.tile_pool(name="v", bufs=40))
        ptpool = ctx.enter_context(tc.tile_pool(name="pt", bufs=6))
        atpool = ctx.enter_context(tc.tile_pool(name="at", bufs=2))
        smpool = ctx.enter_context(tc.tile_pool(name="sm", bufs=2))
        opool = ctx.enter_context(tc.tile_pool(name="o", bufs=4))
        # PSUM budget (8 banks): sc 2x[128,1024]=4, acc0+acc1=2, gen 2x[128,512]=2
        ps_sc = ctx.enter_context(tc.tile_pool(name="pssc", bufs=2, space="PSUM"))
        ps_acc = ctx.enter_context(tc.tile_pool(name="psacc", bufs=1, space="PSUM"))
        ps_gen = ctx.enter_context(tc.tile_pool(name="psgen", bufs=2, space="PSUM"))
        dram = ctx.enter_context(tc.tile_pool(name="dram", bufs=1, space="DRAM"))

        # ---- constants / weights into SBUF (one DMA per weight) ----
        wq_a = wpool.tile([128, D], bf16, tag="wqa")
        nc.sync.dma_start(wq_a[:], wq[:])
        wk_a = wpool.tile([128, D], bf16, tag="wka")
        nc.sync.dma_start(wk_a[:], wk[:])
        wv_a = wpool.tile([128, D], bf16, tag="wva")
        nc.sync.dma_start(wv_a[:], wv[:])
        wq_t = [wq_a[:, k * 128:(k + 1) * 128] for k in range(KT)]
        wk_t = [wk_a[:, k * 128:(k + 1) * 128] for k in range(KT)]
        wv_t = [wv_a[:, k * 128:(k + 1) * 128] for k in range(KT)]
        wo_t = wpool.tile([DKC, D], bf16, tag="wo")
        nc.sync.dma_start(wo_t[:], wo[:])
        bq_t = wpool.tile([DKC, 1], f32, tag="bq")
        nc.sync.dma_start(bq_t[:], bq[:])
        bk_t = wpool.tile([DKC, 1], f32, tag="bk")
        nc.sync.dma_start(bk_t[:], bk[:])
        bv_t = wpool.tile([1, DKC], bf16, tag="bv")
        nc.sync.dma_start(bv_t[:], bv[:])
        ones_t = wpool.tile([1, 128], bf16, tag="ones")
        nc.vector.memset(ones_t[:], 1.0)

        partials = [dram.tile([len(sqs) * 512, D], bf16, tag=f"partial{c}",
                              name=f"partial{c}")
                    for c, (_, sqs, _) in enumerate(CHUNKS)]
        rs_outs = [dram.tile([rows, D], bf16, tag=f"rsout{c}",
                             name=f"rsout{c}")
                   for c, (_, _, rows) in enumerate(CHUNKS)]

        # ---------- emission helpers ----------

        def load_x_half(xT, b, h, tag):
            """One DMA for a [128, 8192] half-batch block (8 k-tiles)."""
            xt = xpool.tile([128, 8192], bf16, tag=tag, name=f"{tag}{b}_{h}")
            r0 = (b * 2 + h) * 128
            nc.sync.dma_start(xt[:], xT[r0:r0 + 128, :])
            return xt

        def xt_sl(xt, k, sl):
            return xt[:, k * 1024 + sl.start:k * 1024 + sl.stop]

        def qk_block(xts_halves, w_list, bias_t, dst, blk):
            """One 512-col block of a q/k projection -> dst[:, blk]."""
            xt = xts_halves[blk // 2]
            hsl = slice((blk % 2) * 512, (blk % 2 + 1) * 512)
            ps = ps_gen.tile([128, 512], f32, tag="gen", name="psqk")
            for k in range(KT):
                nc.tensor.matmul(
                    ps[:], lhsT=w_list[k], rhs=xt_sl(xt, k, hsl),
                    start=(k == 0), stop=(k == KT - 1),
                )
            nc.vector.tensor_scalar_add(
                dst[:, blk * 512:(blk + 1) * 512], ps[:], bias_t[:, 0:1])

        def v_block(xts_halves, blk, v_tiles):
            """Four token-tiles [128, 130] of the v projection."""
            xt = xts_halves[blk // 2]
            for mi in range(4):
                c0 = (blk % 2) * 512 + mi * 128
                ps = ps_gen.tile([128, 512], f32, tag="gen", name="psv")
                for k in range(KT):
                    nc.tensor.matmul(
                        ps[:, 0:128],
                        lhsT=xt[:, k * 1024 + c0:k * 1024 + c0 + 128],
                        rhs=wv_t[k], start=(k == 0), stop=False,
                    )
                nc.tensor.matmul(
                    ps[:, 0:128], lhsT=ones_t[0:1, :], rhs=bv_t[:],
                    start=False, stop=True,
                )
                vt = vpool.tile([128, 130], bf16, tag="v")
                nc.vector.tensor_copy(vt[:, 0:64], ps[:, 0:64])
                nc.vector.tensor_copy(vt[:, 65:129], ps[:, 64:128])
                nc.vector.memset(vt[:, 64:65], 1.0)
                nc.vector.memset(vt[:, 129:130], 1.0)
                v_tiles.append(vt)

        def emit_scores_exp(qT_b, kT_b, sq, sk):
            """Packed scores pair + one exp -> pt tile."""
            qs = slice(sq * 512, (sq + 1) * 512)
            ks = slice(sk * 128, (sk + 1) * 128)
            sps = ps_sc.tile([128, 1024], f32, tag="sc")
            # two heads run concurrently in PE rows 0-63 / 64-127
            nc.tensor.matmul(
                sps[:, 0:512], lhsT=kT_b[0:64, ks], rhs=qT_b[0:64, qs],
                start=True, stop=True,
            )
            nc.tensor.matmul(
                sps[:, 512:1024], lhsT=kT_b[64:128, ks], rhs=qT_b[64:128, qs],
                start=True, stop=True,
            )
            pt = ptpool.tile([128, 1024], bf16, tag="pt")
            nc.scalar.activation(pt[:], sps[:], Act.Exp, scale=0.125)
            return pt

        def emit_pv(v_tiles, accs, pt, sk):
            nc.tensor.matmul(
                accs[0][:], lhsT=v_tiles[sk][:, 0:65], rhs=pt[:, 0:512],
                start=(sk == 0), stop=(sk == SKT - 1),
            )
            nc.tensor.matmul(
                accs[1][:], lhsT=v_tiles[sk][:, 65:130], rhs=pt[:, 512:1024],
                start=(sk == 0), stop=(sk == SKT - 1),
            )

        def drain_accs(accs):
            """Inline epilogue part 1 (cheap DVE copies only): pull sums and
            raw attention out of PSUM so the acc banks free up fast."""
            sums = smpool.tile([1, 1024], bf16, tag="sums")
            araw = smpool.tile([128, 512], bf16, tag="araw")
            nc.vector.tensor_copy(sums[0:1, 0:512], accs[0][64:65, :])
            nc.vector.tensor_copy(sums[0:1, 512:1024], accs[1][64:65, :])
            nc.vector.tensor_copy(araw[0:64, :], accs[0][0:64, :])
            nc.vector.tensor_copy(araw[64:128, :], accs[1][0:64, :])
            return sums, araw

        def make_epilogue(attnT_b, sums, araw, b, sq):
            """Deferred epilogue thunks for (b, sq): broadcast-normalize,
            out-projection, and (if chunk-final) the chunk's ReduceScatter."""
            qs = slice(sq * 512, (sq + 1) * 512)

            def s_norm():
                rbs_ps = ps_gen.tile([128, 512], f32, tag="gen", name="rbs_ps")
                # the two broadcast matmuls run concurrently (col tiling)
                nc.tensor.matmul(
                    rbs_ps[0:64, :], lhsT=ones_t[0:1, 0:64],
                    rhs=sums[0:1, 0:512], start=True, stop=True,
                )
                nc.tensor.matmul(
                    rbs_ps[64:128, :], lhsT=ones_t[0:1, 0:64],
                    rhs=sums[0:1, 512:1024], start=True, stop=True,
                )
                rb = smpool.tile([128, 512], f32, tag="rb")
                nc.vector.reciprocal_approx_fast(rb[:], rbs_ps[:])
                nc.vector.tensor_mul(attnT_b[:, qs], araw[:], rb[:])

            def out_m(m):
                def f():
                    col = sq * 512 + m * 128
                    chunk = next(c for c, (bb, sqs, _) in enumerate(CHUNKS)
                                 if bb == b and sq in sqs)
                    srow = CHUNKS[chunk][1].index(sq) * 512 + m * 128
                    for n2 in range(2):
                        ops = ps_gen.tile([128, 512], f32, tag="gen",
                                          name="psout")
                        nc.tensor.matmul(
                            ops[:], lhsT=attnT_b[:, col:col + 128],
                            rhs=wo_t[:, n2 * 512:(n2 + 1) * 512],
                            start=True, stop=True,
                        )
                        ot = opool.tile([128, 512], bf16, tag="ot")
                        nc.vector.tensor_copy(ot[:], ops[:])
                        nc.sync.dma_start(
                            partials[chunk][srow:srow + 128,
                                            n2 * 512:(n2 + 1) * 512],
                            ot[:],
                        )
                return f

            thunks = [s_norm] + [out_m(m) for m in range(4)]
            chunk = next(c for c, (bb, sqs, _) in enumerate(CHUNKS)
                         if bb == b and sq == sqs[-1])\
                if any(bb == b and sq == sqs[-1] for bb, sqs, _ in CHUNKS) else None
            if chunk is not None and collective:
                rows = CHUNKS[chunk][2]
                off = CHUNK_OFF[chunk]

                def rs():
                    nc.gpsimd.collective_compute(
                        "ReduceScatter",
                        mybir.AluOpType.add,
                        replica_groups=[list(range(NCORES))],
                        ins=[partials[chunk].opt()],
                        outs=[rs_outs[chunk].opt()],
                    )
                    nc.sync.dma_start(
                        out_ext[off:off + rows, :], rs_outs[chunk][:])
                thunks.append(rs)
            return thunks

        # ---------- main emission ----------
        qT, kT, attnT = {}, {}, {}
        vt_all = {0: [], 1: []}
        for b in (0, 1):
            kT[b] = qkpool.tile([128, S], bf16, tag="kT", name=f"kT{b}")
            qT[b] = qkpool.tile([128, S], bf16, tag="qT", name=f"qT{b}")
            attnT[b] = atpool.tile([128, S], bf16, tag="attnT",
                                   name=f"attnT{b}")

        # batch 0: first-half x DMAs land before anything else so the
        # first projection blocks can start ~immediately
        xb0 = {"xk": [None, None], "xq": [None, None], "xv": [None, None]}
        xb1 = {"xk": [None, None], "xq": [None, None], "xv": [None, None]}
        xb0["xk"][0] = load_x_half(xkT, 0, 0, "xk")
        xb0["xq"][0] = load_x_half(xqT, 0, 0, "xq")
        xb0["xv"][0] = load_x_half(xvT, 0, 0, "xv")
        xb0["xk"][1] = load_x_half(xkT, 0, 1, "xk")
        xb0["xq"][1] = load_x_half(xqT, 0, 1, "xq")
        xb0["xv"][1] = load_x_half(xvT, 0, 1, "xv")
        qk_block(xb0["xk"], wk_t, bk_t, kT[0], 0)
        qk_block(xb0["xq"], wq_t, bq_t, qT[0], 0)
        v_block(xb0["xv"], 0, vt_all[0])

        def b1_dma(name, xT, h):
            def f():
                xb1[name][h] = load_x_half(xT, 1, h, name)
            return f

        # the rest of b0's projections + all of b1's become deferred thunks
        extra = [
            lambda: qk_block(xb0["xk"], wk_t, bk_t, kT[0], 1),
            lambda: v_block(xb0["xv"], 1, vt_all[0]),
            lambda: qk_block(xb0["xq"], wq_t, bq_t, qT[0], 1),
            lambda: qk_block(xb0["xk"], wk_t, bk_t, kT[0], 2),
            b1_dma("xk", xkT, 0),
            lambda: v_block(xb0["xv"], 2, vt_all[0]),
            lambda: qk_block(xb0["xq"], wq_t, bq_t, qT[0], 2),
            lambda: qk_block(xb0["xk"], wk_t, bk_t, kT[0], 3),
            b1_dma("xk", xkT, 1),
            lambda: v_block(xb0["xv"], 3, vt_all[0]),
            lambda: qk_block(xb0["xq"], wq_t, bq_t, qT[0], 3),
            b1_dma("xq", xqT, 0),
            b1_dma("xv", xvT, 0),
            b1_dma("xq", xqT, 1),
            b1_dma("xv", xvT, 1),
        ]
        for blk in range(SQB):
            extra.append(lambda blk=blk: qk_block(
                xb1["xk"], wk_t, bk_t, kT[1], blk))
        for blk in range(SQB):
            extra.append(lambda blk=blk: qk_block(
                xb1["xq"], wq_t, bq_t, qT[1], blk))
        for blk in range(SQB):
            extra.append(lambda blk=blk: v_block(xb1["xv"], blk, vt_all[1]))

        pending = []

        def emit_batch_attention(b):
            for sq in range(SQB):
                accs = (
                    ps_acc.tile([65, 512], f32, tag="acc0", name="acc0"),
                    ps_acc.tile([65, 512], f32, tag="acc1", name="acc1"),
                )
                held = None
                for sk in range(SKT):
                    pt = emit_scores_exp(qT[b], kT[b], sq, sk)
                    if held is not None:
                        emit_pv(vt_all[b], accs, *held)
                    held = (pt, sk)
                    if sk >= SKT - 2:
                        continue  # keep the boundary clean for the drain
                    if pending:
                        pending.pop(0)()
                    elif extra:
                        extra.pop(0)()
                emit_pv(vt_all[b], accs, *held)
                sums, araw = drain_accs(accs)
                pending.extend(make_epilogue(attnT[b], sums, araw, b, sq))

        emit_batch_attention(0)
        # all of b1's projection thunks must be emitted before b1's
        # attention reads their outputs (deps are tracked in program order)
        while extra:
            extra.pop(0)()
        emit_batch_attention(1)
        for f in pending:
            f()
        pending.clear()
        while extra:
            extra.pop(0)()

        if not collective:
            for c, (_, _, rows) in enumerate(CHUNKS):
                nc.sync.dma_start(
                    out_ext[CHUNK_OFF[c]:CHUNK_OFF[c] + rows, :],
                    partials[c][0:rows, :],
                )

    nc.compile()
    return nc


def _get_nc():
    if "nc" not in _cache:
        _cache["nc"] = _build()
    return _cache["nc"]


def kernel(query, key, value, Wq, bq, Wk, bk, Wv, bv, Wo, bo, trace=False):
    from concourse.bass_utils import run_bass_kernel_spmd
    import ml_dtypes

    bfloat16 = ml_dtypes.bfloat16
    nc = _get_nc()

    def arrange_x(x):
        # [TOK, D] -> [4, 128, 8192]: row block b*2+h holds tokens
        # [b*2048+h*1024 : +1024), cols = k-tile * 1024 + token offset
        x = np.asarray(x, np.float32).reshape(2, 2, 1024, 8, 128)
        return np.ascontiguousarray(
            x.transpose(0, 1, 4, 3, 2).reshape(512, 8192).astype(bfloat16))

    def arrange_w(w):
        # [D, 128] -> [128, 8*128]: k-tiles side by side
        return np.ascontiguousarray(
            w.reshape(8, 128, 128).transpose(1, 0, 2).reshape(128, 1024)
            .astype(bfloat16))

    q = arrange_x(query)
    k = arrange_x(key)
    v = arrange_x(value)
    Wq = np.asarray(Wq, np.float32)
    Wk = np.asarray(Wk, np.float32)
    Wv = np.asarray(Wv, np.float32)
    Wo = np.asarray(Wo, np.float32)

    in_maps = []
    for r in range(NCORES):
        sl = slice(r * DKC, (r + 1) * DKC)
        in_maps.append({
            "xqT": q, "xkT": k, "xvT": v,
            "wq": arrange_w(Wq[:, sl]),
            "wk": arrange_w(Wk[:, sl]),
            "wv": arrange_w(Wv[:, sl]),
            "wo": np.ascontiguousarray(Wo[sl, :].astype(bfloat16)),
            "bq": np.ascontiguousarray(np.asarray(bq, np.float32)[sl, None]),
            "bk": np.ascontiguousarray(np.asarray(bk, np.float32)[sl, None]),
            "bv": np.ascontiguousarray(
                np.asarray(bv, np.float32)[None, sl].astype(bfloat16)),
        })

    res = run_bass_kernel_spmd(nc, in_maps, list(range(NCORES)), trace=trace)
    _cache["last_results"] = res

    # Reassemble: chunk c scatters its rows over cores; core r's piece of
    # chunk c sits at out_ext[CHUNK_OFF[c] : +rows].
    out = np.zeros((TOK, D), np.float32)
    for r in range(NCORES):
        o = np.asarray(res.results[r]["out"]).astype(np.float32)
        for c, (b, sqs, rows) in enumerate(CHUNKS):
            t0 = b * S + sqs[0] * 512 + r * rows
            off = CHUNK_OFF[c]
            out[t0:t0 + rows] = o[off:off + rows]
    out = out + np.asarray(bo, np.float32)[None, :]
    return out.reshape(B, S, D)
